# revision 10
# baseline (speedup 1.0000x reference)
"""Multi-head causal self-attention (B=4, T=2048, C=1024, H=16) on 8 TRN2 cores.

Sharding: core pair {2b, 2b+1} owns batch b; even core computes heads 0-7,
odd core heads 8-15 (tensor parallel over heads). Each core:
  1. qkvT projection from host-packed xT (bf16 matmuls, fp32 PSUM)
  2. causal attention in scoresT [Tk, Tq] orientation, Tq-chunk-outer:
     two heads' score matmuls packed into disjoint PE row groups, one exp
     per pair on ScalarE (scale=1/8), diagonal tiles N-restricted to the
     causally-live q range with a single [128,128] triangle mask multiply,
     AV^T matmuls with an appended ones column producing softmax
     denominators for free
  3. per-chunk softmax normalization: [8,512] reciprocals + K=8 selector
     matmul broadcast + in-place multiplies
  4. partial c_proj over local 512 channels; bias (v-bias + proj bias,
     halved) added during the PSUM drain from a precomputed broadcast tile
  5. every proj tile DMAs straight to DRAM; NO collectives — the host sums
     the two partials of each core pair (c_proj input-row reduction).
Host packs inputs so every DMA moves >=1KB contiguous per-partition lines.
"""

import math
import numpy as np
import ml_dtypes
from contextlib import ExitStack

import concourse.bass as bass
import concourse.tile as tile
from concourse import bacc, mybir
from concourse.bass_utils import run_bass_kernel_spmd

bf16 = ml_dtypes.bfloat16
F32 = mybir.dt.float32
BF16 = mybir.dt.bfloat16
AF = mybir.ActivationFunctionType
ADD = mybir.AluOpType.add

B, T, C, H = 4, 2048, 1024, 16
D = C // H              # 64 head dim
NCORES = 8
HL = H // 2             # 8 heads per core
CL = HL * D             # 512 local channels

_CACHE = {}


def _build():
    nc = bacc.Bacc("TRN2", target_bir_lowering=False, debug=False,
                   num_devices=NCORES)

    xT_d = nc.dram_tensor("xT", [4, 128, 4096], BF16, kind="ExternalInput").ap()
    waq_d = nc.dram_tensor("waq", [128, 4, 1024], BF16, kind="ExternalInput").ap()
    wak_d = nc.dram_tensor("wak", [128, 4, 1024], BF16, kind="ExternalInput").ap()
    wav_d = nc.dram_tensor("wav", [128, 8, 512], BF16, kind="ExternalInput").ap()
    wp_d = nc.dram_tensor("wp", [128, 4, 1024], BF16, kind="ExternalInput").ap()
    bqk_d = nc.dram_tensor("bqk", [128, 8], F32, kind="ExternalInput").ap()
    bye_d = nc.dram_tensor("bye", [1, C], BF16, kind="ExternalInput").ap()
    um_d = nc.dram_tensor("um", [128, 128], BF16, kind="ExternalInput").ap()
    sel_d = nc.dram_tensor("sel", [4, 8, 128], BF16, kind="ExternalInput").ap()
    y_d = nc.dram_tensor("y", [16, 128, C], BF16, kind="ExternalOutput").ap()

    with tile.TileContext(nc) as tc, ExitStack() as ctx:
        cst = ctx.enter_context(tc.tile_pool(name="cst", bufs=1))
        work = ctx.enter_context(tc.tile_pool(name="work", bufs=6))
        ysb_p = ctx.enter_context(tc.tile_pool(name="ysb", bufs=2))
        stg_p = ctx.enter_context(tc.tile_pool(name="stg", bufs=4))
        ps_mm = ctx.enter_context(tc.tile_pool(name="psmm", bufs=2, space="PSUM"))
        ps_s = ctx.enter_context(tc.tile_pool(name="pss", bufs=2, space="PSUM"))
        ps_av = ctx.enter_context(tc.tile_pool(name="psav", bufs=2, space="PSUM"))

        # ---- persistent SBUF tensors ----
        xT = cst.tile([128, 8, T], BF16)        # x^T  (C on partitions)
        waq = cst.tile([128, 4, 8, 128], BF16)  # W_q col-slice, j-major
        wak = cst.tile([128, 4, 8, 128], BF16)  # W_k col-slice, j-major
        wav = cst.tile([128, 8, 512], BF16)     # W_v col-slice
        wp = cst.tile([128, 4, C], BF16)        # W_proj local rows
        bqk = cst.tile([128, 8], F32)           # q/k biases per qT/kT tile
        bye = cst.tile([1, C], BF16)            # (b_proj + bv@W_proj)/2
        bye_bc = cst.tile([128, C], BF16)       # bye broadcast to 128 rows
        um = cst.tile([128, 1, 128], BF16)      # causal triangle p<=f
        ones = cst.tile([1, 128], BF16)
        sel = cst.tile([8, 4, 128], BF16)       # per-j K=8 broadcast selectors
        qT = cst.tile([128, 4, T], BF16)
        kT = cst.tile([128, 4, T], BF16)
        vaug = cst.tile([128, 16, HL, D + 1], BF16)  # v rows + ones col
        avT = cst.tile([128, 4, T], BF16)       # attn-out^T (raw, then normed)
        sums = cst.tile([8, 4, 512], F32)       # LN of softmax denominators
        rsum = cst.tile([8, 4, 512], BF16)      # reciprocals exp(-ln)

        # ---- input DMAs: small tensors + first-needed blocks first ----
        xT_r = xT.rearrange("p ko (ch t) -> p ch ko t", ch=4)
        xT_dr = xT_d.rearrange("ch p (ko t) -> ch p ko t", ko=8)
        nc.scalar.dma_start(xT_r[:, 0], xT_dr[0])
        nc.gpsimd.dma_start(waq[:, 0], waq_d[:, 0].rearrange("p (ko n) -> p ko n", n=128))
        nc.gpsimd.dma_start(wak[:, 0], wak_d[:, 0].rearrange("p (ko n) -> p ko n", n=128))
        nc.gpsimd.dma_start(bqk[:], bqk_d)
        nc.gpsimd.dma_start(bye[:], bye_d)
        nc.gpsimd.dma_start(um[:, 0:1, :], um_d)
        nc.gpsimd.dma_start(sel[:], sel_d.rearrange("j k p -> k j p"))
        nc.gpsimd.dma_start(wav[:], wav_d)
        for ch in range(1, 4):
            nc.scalar.dma_start(xT_r[:, ch], xT_dr[ch])
        for j in range(1, 4):
            nc.gpsimd.dma_start(
                waq[:, j], waq_d[:, j].rearrange("p (ko n) -> p ko n", n=128))
            nc.gpsimd.dma_start(
                wak[:, j], wak_d[:, j].rearrange("p (ko n) -> p ko n", n=128))
        nc.gpsimd.dma_start(wp[:], wp_d)
        nc.vector.memset(ones[:], 1.0)
        nc.vector.memset(vaug[:, :, :, D], 1.0)

        # ---- QKV projection pieces ----
        # qT/kT tile j holds heads {2j, 2j+1}.  qT = W_q^T @ x^T.
        def qk_chain(qk, j, ch):
            dst, w = (qT, waq) if qk == 0 else (kT, wak)
            ps = ps_mm.tile([128, 512], F32, tag="mm")
            for ko in range(8):
                nc.tensor.matmul(
                    ps[:],
                    lhsT=w[:, j, ko],
                    rhs=xT[:, ko, 512 * ch:512 * (ch + 1)],
                    start=(ko == 0), stop=(ko == 7))
            nc.vector.tensor_tensor(
                dst[:, j, 512 * ch:512 * (ch + 1)], ps[:],
                bqk[:, 4 * qk + j:4 * qk + j + 1].to_broadcast((128, 512)),
                ADD)

        def qk_proj(qk, j):
            for ch in range(4):
                qk_chain(qk, j, ch)

        # v in natural [T, c_local] layout, interleaved with ones columns
        def v_chain(m):
            ps = ps_mm.tile([128, 512], F32, tag="mm")
            for ko in range(8):
                nc.tensor.matmul(
                    ps[:],
                    lhsT=xT[:, ko, 128 * m:128 * (m + 1)],
                    rhs=wav[:, ko],
                    start=(ko == 0), stop=(ko == 7))
            nc.vector.tensor_copy(
                vaug[:, m, :, 0:D],
                ps[:].rearrange("p (h d) -> p h d", d=D))

        # bye broadcast tile: one K=1 matmul pair + drains (run once early)
        def build_bye_bc():
            for n in range(2):
                ps = ps_mm.tile([128, 512], F32, tag="mm")
                nc.tensor.matmul(ps[:], lhsT=ones[0:1, :],
                                 rhs=bye[0:1, 512 * n:512 * (n + 1)],
                                 start=True, stop=True)
                nc.vector.tensor_copy(bye_bc[:, 512 * n:512 * (n + 1)], ps[:])

        def attn(j, c, fillers=()):
            """Head pair {2j, 2j+1}, Tq chunk c; row-group packed scores.
            Diagonal k-tiles (m >= 4c) restrict scores/exp/AV to the live
            q-range [128s, 512) and mask only the 128-col diagonal block.
            fillers: closures emitted mid-loop to keep PE fed during exp waits.
            """
            fillers = list(fillers)
            ntk = 4 * (c + 1)
            pavA = ps_av.tile([D + 1, 512], F32, tag="av")
            pavB = ps_av.tile([D + 1, 512], F32, tag="av")
            for m in range(ntk):
                s = m - 4 * c
                lo = 128 * s if s > 0 else 0
                pss = ps_s.tile([128, 2, 512], F32, tag="s")
                for hh in range(2):
                    ro = hh * 64
                    nc.tensor.matmul(
                        pss[:, hh, lo:512],
                        lhsT=kT[ro:ro + 64, j, 128 * m:128 * (m + 1)],
                        rhs=qT[ro:ro + 64, j, 512 * c + lo:512 * (c + 1)],
                        start=True, stop=True)
                ex = work.tile([128, 2, 512], BF16, tag="expT")
                nc.scalar.activation(ex[:, :, lo:512], pss[:, :, lo:512],
                                     AF.Exp, scale=1.0 / math.sqrt(D))
                if s >= 0:
                    nc.vector.tensor_mul(
                        ex[:, :, lo:lo + 128], ex[:, :, lo:lo + 128],
                        um[:, 0:1, :].to_broadcast((128, 2, 128)))
                for hh in range(2):
                    nc.tensor.matmul(
                        (pavA if hh == 0 else pavB)[:, lo:512],
                        lhsT=vaug[:, m, 2 * j + hh, :],
                        rhs=ex[:, hh, lo:512],
                        start=(m == 0), stop=(m == ntk - 1))
                if fillers and m + 1 >= max(1, ntk - 2 * len(fillers)) \
                        and (m & 1):
                    fillers.pop(0)()
            while fillers:
                fillers.pop(0)()
            for hh in range(2):
                h = 2 * j + hh
                ro = hh * 64
                pav = pavA if hh == 0 else pavB
                nc.vector.tensor_copy(
                    avT[ro:ro + 64, j, 512 * c:512 * (c + 1)], pav[0:D, :])
                stg = stg_p.tile([1, 512], F32, tag="stg")
                nc.scalar.activation(stg[:], pav[D:D + 1, :], AF.Ln)
                nc.sync.dma_start(sums[h:h + 1, c, :], stg[:])

        def rsum_make(c):
            """Whole-chunk softmax reciprocals in one ACT op: exp(-ln(sum))."""
            nc.scalar.activation(rsum[:, c, :], sums[:, c, :], AF.Exp,
                                 scale=-1.0)

        def norm_mult(c, j):
            pbc = ps_mm.tile([128, 512], F32, tag="mm")
            nc.tensor.matmul(pbc[:], lhsT=sel[:, j, :], rhs=rsum[:, c, :],
                             start=True, stop=True)
            nc.vector.tensor_mul(
                avT[:, j, 512 * c:512 * (c + 1)],
                avT[:, j, 512 * c:512 * (c + 1)], pbc[:])

        def norm_fillers(c):
            return [lambda c=c, j=j: norm_mult(c, j) for j in range(4)]

        def norm(c):
            for f in norm_fillers(c):
                f()

        ysb_tiles = {}

        def proj_chain(mt, n):
            """One 512-col n-chunk of c_proj for T-tile mt (+ DMA on n=1)."""
            if n == 0:
                ysb_tiles[mt] = ysb_p.tile([128, C], BF16, tag="y",
                                           name=f"ysb{mt}")
            ysb = ysb_tiles[mt]
            ps = ps_mm.tile([128, 512], F32, tag="mm")
            for j2 in range(4):
                nc.tensor.matmul(
                    ps[:],
                    lhsT=avT[:, j2, 128 * mt:128 * (mt + 1)],
                    rhs=wp[:, j2, 512 * n:512 * (n + 1)],
                    start=(j2 == 0), stop=(j2 == 3))
            nc.vector.tensor_tensor(
                ysb[:, 512 * n:512 * (n + 1)], ps[:],
                bye_bc[:, 512 * n:512 * (n + 1)], ADD)
            if n == 1:
                del ysb_tiles[mt]
                nc.gpsimd.dma_start(y_d[mt], ysb[:])

        def proj_tile(mt):
            proj_chain(mt, 0)
            proj_chain(mt, 1)

        # ---- global schedule: chunks processed 3 -> 2 -> 1 -> 0 ----
        # QKV interleaved with chunk-3 attention; each chunk's proj tiles
        # interleave the next chunk's attention; all proj tiles DMA straight
        # to DRAM (host sums the core-pair partials).
        qk_chain(0, 0, 0)
        qk_chain(1, 0, 0)
        build_bye_bc()
        for m in range(4):
            v_chain(m)
        for ch in range(1, 4):
            qk_chain(0, 0, ch)
            qk_chain(1, 0, ch)
            for m in range(4 * ch, 4 * ch + 4):
                v_chain(m)
        attn(0, 3)
        for j in range(1, 4):
            qk_proj(0, j)
            qk_proj(1, j)
            attn(j, 3)

        def pf(mt, n):
            return lambda: proj_chain(mt, n)

        # chunk order 3,2,1,0; norm(c) deferred into the next chunk's first
        # attention; chunk c's proj chains filled into attns j>=1 of the next
        rsum_make(3)
        attn(0, 2, fillers=norm_fillers(3) + [pf(12, 0)])
        attn(1, 2, fillers=[pf(12, 1), pf(13, 0), pf(13, 1)])
        attn(2, 2, fillers=[pf(14, 0), pf(14, 1), pf(15, 0)])
        attn(3, 2, fillers=[pf(15, 1)])
        rsum_make(2)
        attn(0, 1, fillers=norm_fillers(2) + [pf(8, 0)])
        attn(1, 1, fillers=[pf(8, 1), pf(9, 0), pf(9, 1)])
        attn(2, 1, fillers=[pf(10, 0), pf(10, 1), pf(11, 0)])
        attn(3, 1, fillers=[pf(11, 1)])
        rsum_make(1)
        attn(0, 0, fillers=norm_fillers(1) + [pf(4, 0)])
        attn(1, 0, fillers=[pf(4, 1), pf(5, 0), pf(5, 1)])
        attn(2, 0, fillers=[pf(6, 0), pf(6, 1), pf(7, 0)])
        attn(3, 0, fillers=[pf(7, 1)])
        rsum_make(0)
        norm(0)
        for mt in range(4):
            proj_tile(mt)

    nc.compile()
    return nc


def _prep_inputs(x, W_attn, b_attn, W_proj, b_proj):
    x = np.asarray(x, dtype=np.float32)
    W_attn = np.asarray(W_attn, dtype=np.float32)
    b_attn = np.asarray(b_attn, dtype=np.float32)
    W_proj = np.asarray(W_proj, dtype=np.float32)
    b_proj = np.asarray(b_proj, dtype=np.float32)

    bv = b_attn[2 * C:3 * C]
    bye_full = (b_proj + bv @ W_proj) * 0.5
    bye = np.ascontiguousarray(bye_full[None, :]).astype(bf16)

    # causal triangle for the 128-col diagonal block: keep iff p <= f
    um = (np.arange(128)[:, None] <= np.arange(128)[None, :]).astype(bf16)

    # K=8 broadcast selectors: out partition p gets rsum row 2j + (p >= 64)
    sel = np.zeros((4, 8, 128), np.float32)
    for j in range(4):
        sel[j, 2 * j, 0:64] = 1.0
        sel[j, 2 * j + 1, 64:128] = 1.0
    sel = sel.astype(bf16)

    in_maps = []
    for c in range(NCORES):
        b, r = c // 2, c % 2
        # xT packed: [ch, p, ko*512+t'] = x[b, 512ch+t', 128ko+p]
        xT = np.ascontiguousarray(
            x[b].reshape(4, 512, 8, 128).transpose(0, 3, 2, 1)
        ).reshape(4, 128, 4096).astype(bf16)
        qs, ks, vs = CL * r, C + CL * r, 2 * C + CL * r
        # waq/wak: [p, j, ko*128+n'] = W[:, 128j+n'] row 128ko+p
        waq = np.ascontiguousarray(
            W_attn[:, qs:qs + CL].reshape(8, 128, 4, 128).transpose(1, 2, 0, 3)
        ).reshape(128, 4, 1024).astype(bf16)
        wak = np.ascontiguousarray(
            W_attn[:, ks:ks + CL].reshape(8, 128, 4, 128).transpose(1, 2, 0, 3)
        ).reshape(128, 4, 1024).astype(bf16)
        wav = np.ascontiguousarray(
            W_attn[:, vs:vs + CL].reshape(8, 128, 512).transpose(1, 0, 2)
        ).astype(bf16)
        wp = np.ascontiguousarray(
            W_proj[CL * r:CL * (r + 1), :].reshape(4, 128, C).transpose(1, 0, 2)
        ).astype(bf16)
        bqk = np.empty((128, 8), np.float32)
        for j in range(4):
            bqk[:, j] = b_attn[qs + 128 * j:qs + 128 * (j + 1)]
            bqk[:, 4 + j] = b_attn[ks + 128 * j:ks + 128 * (j + 1)]
        in_maps.append({"xT": np.asarray(xT), "waq": np.asarray(waq),
                        "wak": np.asarray(wak), "wav": np.asarray(wav),
                        "wp": np.asarray(wp), "bqk": bqk,
                        "bye": np.asarray(bye), "um": np.asarray(um),
                        "sel": np.asarray(sel)})
    return in_maps


def kernel(x, W_attn, b_attn, W_proj, b_proj, _trace=False, _result=[None]):
    if "nc" not in _CACHE:
        _CACHE["nc"] = _build()
    nc = _CACHE["nc"]
    in_maps = _prep_inputs(x, W_attn, b_attn, W_proj, b_proj)
    res = run_bass_kernel_spmd(nc, in_maps, list(range(NCORES)), trace=_trace)
    _result[0] = res
    out = np.empty((B, T, C), np.float32)
    for b in range(B):
        ya = res.results[2 * b]["y"].astype(np.float32)
        yb = res.results[2 * b + 1]["y"].astype(np.float32)
        out[b] = (ya + yb).reshape(T, C)
    return out


# revision 15
# speedup vs baseline: 1.3650x; 1.3650x over previous
"""Multi-head causal self-attention (B=4, T=2048, C=1024, H=16) on 8 TRN2 cores.

Sharding: core pair {2b, 2b+1} owns batch b; even core computes heads 0-7,
odd core heads 8-15 (tensor parallel over heads). Each core:
  1. qkvT projection from host-packed xT (bf16 matmuls, fp32 PSUM)
  2. causal attention in scoresT [Tk, Tq] orientation, Tq-chunk-outer:
     two heads' score matmuls packed into disjoint PE row groups, one exp
     per pair on ScalarE (scale=1/8), diagonal tiles N-restricted to the
     causally-live q range with a single [128,128] triangle mask multiply,
     AV^T matmuls with an appended ones column producing softmax
     denominators for free
  3. per-chunk softmax normalization: [8,512] reciprocals + K=8 selector
     matmul broadcast + in-place multiplies
  4. partial c_proj over local 512 channels; bias (v-bias + proj bias,
     halved) added during the PSUM drain from a precomputed broadcast tile
  5. every proj tile DMAs straight to DRAM; NO collectives — the host sums
     the two partials of each core pair (c_proj input-row reduction).
Host packs inputs so every DMA moves >=1KB contiguous per-partition lines.
"""

import math
import numpy as np
import ml_dtypes
from contextlib import ExitStack

import concourse.bass as bass
import concourse.tile as tile
from concourse import bacc, mybir
from concourse.bass_utils import run_bass_kernel_spmd

bf16 = ml_dtypes.bfloat16
F32 = mybir.dt.float32
BF16 = mybir.dt.bfloat16
AF = mybir.ActivationFunctionType
ADD = mybir.AluOpType.add

B, T, C, H = 4, 2048, 1024, 16
D = C // H              # 64 head dim
NCORES = 8
HL = H // 2             # 8 heads per core
CL = HL * D             # 512 local channels

_CACHE = {}


def _build():
    nc = bacc.Bacc("TRN2", target_bir_lowering=False, debug=False,
                   num_devices=NCORES)

    xT_d = nc.dram_tensor("xT", [4, 128, 4096], BF16, kind="ExternalInput").ap()
    waq_d = nc.dram_tensor("waq", [128, 4, 1024], BF16, kind="ExternalInput").ap()
    wak_d = nc.dram_tensor("wak", [128, 4, 1024], BF16, kind="ExternalInput").ap()
    wav_d = nc.dram_tensor("wav", [128, 8, 512], BF16, kind="ExternalInput").ap()
    wp_d = nc.dram_tensor("wp", [128, 4, 1024], BF16, kind="ExternalInput").ap()
    bqk_d = nc.dram_tensor("bqk", [128, 8], F32, kind="ExternalInput").ap()
    bye_d = nc.dram_tensor("bye", [1, C], BF16, kind="ExternalInput").ap()
    um_d = nc.dram_tensor("um", [128, 128], BF16, kind="ExternalInput").ap()
    sel_d = nc.dram_tensor("sel", [4, 8, 128], BF16, kind="ExternalInput").ap()
    y_d = nc.dram_tensor("y", [16, 128, C], BF16, kind="ExternalOutput").ap()

    with tile.TileContext(nc) as tc, ExitStack() as ctx:
        cst = ctx.enter_context(tc.tile_pool(name="cst", bufs=1))
        work = ctx.enter_context(tc.tile_pool(name="work", bufs=6))
        ysb_p = ctx.enter_context(tc.tile_pool(name="ysb", bufs=2))
        stg_p = ctx.enter_context(tc.tile_pool(name="stg", bufs=4))
        ps_mm = ctx.enter_context(tc.tile_pool(name="psmm", bufs=2, space="PSUM"))
        ps_s = ctx.enter_context(tc.tile_pool(name="pss", bufs=2, space="PSUM"))
        ps_av = ctx.enter_context(tc.tile_pool(name="psav", bufs=2, space="PSUM"))

        # ---- persistent SBUF tensors ----
        xT = cst.tile([128, 8, T], BF16)        # x^T  (C on partitions)
        waq = cst.tile([128, 4, 8, 128], BF16)  # W_q col-slice, j-major
        wak = cst.tile([128, 4, 8, 128], BF16)  # W_k col-slice, j-major
        wav = cst.tile([128, 8, 512], BF16)     # W_v col-slice
        wp = cst.tile([128, 4, C], BF16)        # W_proj local rows
        bqk = cst.tile([128, 8], F32)           # q/k biases per qT/kT tile
        bye = cst.tile([1, C], BF16)            # (b_proj + bv@W_proj)/2
        bye_bc = cst.tile([128, C], BF16)       # bye broadcast to 128 rows
        um = cst.tile([128, 1, 128], BF16)      # causal triangle p<=f
        ones = cst.tile([1, 128], BF16)
        sel = cst.tile([8, 4, 128], BF16)       # per-j K=8 broadcast selectors
        qT = cst.tile([128, 4, T], BF16)
        kT = cst.tile([128, 4, T], BF16)
        vaug = cst.tile([128, 16, HL, D + 1], BF16)  # v rows + ones col
        avT = cst.tile([128, 4, T], BF16)       # attn-out^T (raw, then normed)
        sums = cst.tile([8, 4, 512], F32)       # softmax denominators [h, c, tq]
        rsc = cst.tile([8, 512], F32)           # fp32 reciprocal scratch
        rsum = cst.tile([8, 4, 512], BF16)      # their reciprocals

        # ---- input DMAs: small tensors + first-needed blocks first ----
        xT_r = xT.rearrange("p ko (ch t) -> p ch ko t", ch=4)
        xT_dr = xT_d.rearrange("ch p (ko t) -> ch p ko t", ko=8)
        nc.scalar.dma_start(xT_r[:, 0], xT_dr[0])
        nc.gpsimd.dma_start(waq[:, 0], waq_d[:, 0].rearrange("p (ko n) -> p ko n", n=128))
        nc.gpsimd.dma_start(wak[:, 0], wak_d[:, 0].rearrange("p (ko n) -> p ko n", n=128))
        nc.gpsimd.dma_start(bqk[:], bqk_d)
        nc.gpsimd.dma_start(bye[:], bye_d)
        nc.gpsimd.dma_start(um[:, 0:1, :], um_d)
        nc.gpsimd.dma_start(sel[:], sel_d.rearrange("j k p -> k j p"))
        nc.gpsimd.dma_start(wav[:], wav_d)
        for ch in range(1, 4):
            nc.scalar.dma_start(xT_r[:, ch], xT_dr[ch])
        for j in range(1, 4):
            nc.gpsimd.dma_start(
                waq[:, j], waq_d[:, j].rearrange("p (ko n) -> p ko n", n=128))
            nc.gpsimd.dma_start(
                wak[:, j], wak_d[:, j].rearrange("p (ko n) -> p ko n", n=128))
        nc.gpsimd.dma_start(wp[:], wp_d)
        nc.vector.memset(ones[:], 1.0)
        nc.vector.memset(vaug[:, :, :, D], 1.0)

        # ---- QKV projection pieces ----
        # qT/kT tile j holds heads {2j, 2j+1}.  qT = W_q^T @ x^T.
        def qk_chain(qk, j, ch):
            dst, w = (qT, waq) if qk == 0 else (kT, wak)
            ps = ps_mm.tile([128, 512], F32, tag="mm")
            for ko in range(8):
                nc.tensor.matmul(
                    ps[:],
                    lhsT=w[:, j, ko],
                    rhs=xT[:, ko, 512 * ch:512 * (ch + 1)],
                    start=(ko == 0), stop=(ko == 7))
            nc.vector.tensor_tensor(
                dst[:, j, 512 * ch:512 * (ch + 1)], ps[:],
                bqk[:, 4 * qk + j:4 * qk + j + 1].to_broadcast((128, 512)),
                ADD)

        def qk_proj(qk, j):
            for ch in range(4):
                qk_chain(qk, j, ch)

        # v in natural [T, c_local] layout, interleaved with ones columns
        def v_chain(m):
            ps = ps_mm.tile([128, 512], F32, tag="mm")
            for ko in range(8):
                nc.tensor.matmul(
                    ps[:],
                    lhsT=xT[:, ko, 128 * m:128 * (m + 1)],
                    rhs=wav[:, ko],
                    start=(ko == 0), stop=(ko == 7))
            nc.vector.tensor_copy(
                vaug[:, m, :, 0:D],
                ps[:].rearrange("p (h d) -> p h d", d=D))

        # bye broadcast tile: one K=1 matmul pair + drains (run once early)
        def build_bye_bc():
            for n in range(2):
                ps = ps_mm.tile([128, 512], F32, tag="mm")
                nc.tensor.matmul(ps[:], lhsT=ones[0:1, :],
                                 rhs=bye[0:1, 512 * n:512 * (n + 1)],
                                 start=True, stop=True)
                nc.vector.tensor_copy(bye_bc[:, 512 * n:512 * (n + 1)], ps[:])

        def attn(j, c, fillers=()):
            """Head pair {2j, 2j+1}, Tq chunk c; row-group packed scores.
            Diagonal k-tiles (m >= 4c) restrict scores/exp/AV to the live
            q-range [128s, 512) and mask only the 128-col diagonal block.
            fillers: closures emitted mid-loop to keep PE fed during exp waits.
            """
            fillers = list(fillers)
            ntk = 4 * (c + 1)
            pavA = ps_av.tile([D + 1, 512], F32, tag="av")
            pavB = ps_av.tile([D + 1, 512], F32, tag="av")
            for m in range(ntk):
                s = m - 4 * c
                lo = 128 * s if s > 0 else 0
                pss = ps_s.tile([128, 2, 512], F32, tag="s")
                for hh in range(2):
                    ro = hh * 64
                    nc.tensor.matmul(
                        pss[:, hh, lo:512],
                        lhsT=kT[ro:ro + 64, j, 128 * m:128 * (m + 1)],
                        rhs=qT[ro:ro + 64, j, 512 * c + lo:512 * (c + 1)],
                        start=True, stop=True)
                ex = work.tile([128, 2, 512], BF16, tag="expT")
                nc.scalar.activation(ex[:, :, lo:512], pss[:, :, lo:512],
                                     AF.Exp, scale=1.0 / math.sqrt(D))
                if s >= 0:
                    nc.vector.tensor_mul(
                        ex[:, :, lo:lo + 128], ex[:, :, lo:lo + 128],
                        um[:, 0:1, :].to_broadcast((128, 2, 128)))
                for hh in range(2):
                    nc.tensor.matmul(
                        (pavA if hh == 0 else pavB)[:, lo:512],
                        lhsT=vaug[:, m, 2 * j + hh, :],
                        rhs=ex[:, hh, lo:512],
                        start=(m == 0), stop=(m == ntk - 1))
                if fillers and m + 1 >= max(1, ntk - 2 * len(fillers)) \
                        and (m & 1):
                    fillers.pop(0)()
            while fillers:
                fillers.pop(0)()
            # free the pav banks first (both avT copies), then the stg rows
            for hh in range(2):
                ro = hh * 64
                pav = pavA if hh == 0 else pavB
                nc.vector.tensor_copy(
                    avT[ro:ro + 64, j, 512 * c:512 * (c + 1)], pav[0:D, :])
            for hh in range(2):
                h = 2 * j + hh
                pav = pavA if hh == 0 else pavB
                stg = stg_p.tile([1, 512], F32, tag="stg")
                nc.vector.tensor_copy(stg[:], pav[D:D + 1, :])
                nc.sync.dma_start(sums[h:h + 1, c, :], stg[:])

        def recip_fast(c):
            """Chunk reciprocals in one custom-DVE op (~51 ULP, plenty)."""
            nc.vector.reciprocal_approx_fast(rsc[:], sums[:, c, :])

        def recip_cast(c):
            with nc.allow_low_precision(reason="softmax reciprocal in bf16"):
                nc.vector.tensor_copy(rsum[:, c, :], rsc[:])

        def norm_mult(c, j):
            pbc = ps_mm.tile([128, 512], F32, tag="mm")
            nc.tensor.matmul(pbc[:], lhsT=sel[:, j, :], rhs=rsum[:, c, :],
                             start=True, stop=True)
            nc.vector.tensor_mul(
                avT[:, j, 512 * c:512 * (c + 1)],
                avT[:, j, 512 * c:512 * (c + 1)], pbc[:])

        def norm_fillers(c):
            return ([lambda c=c: recip_fast(c), lambda c=c: recip_cast(c)]
                    + [lambda c=c, j=j: norm_mult(c, j) for j in range(4)])

        def norm(c):
            for f in norm_fillers(c):
                f()

        ysb_tiles = {}

        def proj_chain(mt, n):
            """One 512-col n-chunk of c_proj for T-tile mt (+ DMA on n=1)."""
            if n == 0:
                ysb_tiles[mt] = ysb_p.tile([128, C], BF16, tag="y",
                                           name=f"ysb{mt}")
            ysb = ysb_tiles[mt]
            ps = ps_mm.tile([128, 512], F32, tag="mm")
            for j2 in range(4):
                nc.tensor.matmul(
                    ps[:],
                    lhsT=avT[:, j2, 128 * mt:128 * (mt + 1)],
                    rhs=wp[:, j2, 512 * n:512 * (n + 1)],
                    start=(j2 == 0), stop=(j2 == 3))
            nc.vector.tensor_tensor(
                ysb[:, 512 * n:512 * (n + 1)], ps[:],
                bye_bc[:, 512 * n:512 * (n + 1)], ADD)
            if n == 1:
                del ysb_tiles[mt]
                nc.sync.dma_start(y_d[mt], ysb[:])

        def proj_tile(mt):
            proj_chain(mt, 0)
            proj_chain(mt, 1)

        # ---- global schedule: chunks processed 3 -> 2 -> 1 -> 0 ----
        # QKV interleaved with chunk-3 attention; each chunk's proj tiles
        # interleave the next chunk's attention; all proj tiles DMA straight
        # to DRAM (host sums the core-pair partials).
        qk_chain(0, 0, 0)
        qk_chain(1, 0, 0)
        build_bye_bc()
        for m in range(4):
            v_chain(m)
        for ch in range(1, 4):
            qk_chain(0, 0, ch)
            qk_chain(1, 0, ch)
            for m in range(4 * ch, 4 * ch + 4):
                v_chain(m)
        attn(0, 3)
        for j in range(1, 4):
            qk_proj(0, j)
            qk_proj(1, j)
            attn(j, 3)

        def pf(mt, n):
            return lambda: proj_chain(mt, n)

        # chunk order 3,2,1,0; norm(c) deferred into the next chunk's first
        # attention; chunk c's proj chains filled into attns j>=1 of the next
        attn(0, 2, fillers=norm_fillers(3))
        attn(1, 2, fillers=[pf(12, 0), pf(12, 1), pf(13, 0)])
        attn(2, 2, fillers=[pf(13, 1), pf(14, 0), pf(14, 1)])
        attn(3, 2, fillers=[pf(15, 0), pf(15, 1)])
        attn(0, 1, fillers=norm_fillers(2))
        attn(1, 1, fillers=[pf(8, 0), pf(8, 1), pf(9, 0)])
        attn(2, 1, fillers=[pf(9, 1), pf(10, 0), pf(10, 1)])
        attn(3, 1, fillers=[pf(11, 0), pf(11, 1)])
        attn(0, 0, fillers=norm_fillers(1))
        attn(1, 0, fillers=[pf(4, 0), pf(4, 1), pf(5, 0)])
        attn(2, 0, fillers=[pf(5, 1), pf(6, 0), pf(6, 1)])
        attn(3, 0, fillers=[pf(7, 0), pf(7, 1)])
        norm(0)
        for mt in range(4):
            proj_tile(mt)

    nc.compile()
    return nc


def _prep_inputs(x, W_attn, b_attn, W_proj, b_proj):
    x = np.asarray(x, dtype=np.float32)
    W_attn = np.asarray(W_attn, dtype=np.float32)
    b_attn = np.asarray(b_attn, dtype=np.float32)
    W_proj = np.asarray(W_proj, dtype=np.float32)
    b_proj = np.asarray(b_proj, dtype=np.float32)

    bv = b_attn[2 * C:3 * C]
    bye_full = (b_proj + bv @ W_proj) * 0.5
    bye = np.ascontiguousarray(bye_full[None, :]).astype(bf16)

    # causal triangle for the 128-col diagonal block: keep iff p <= f
    um = (np.arange(128)[:, None] <= np.arange(128)[None, :]).astype(bf16)

    # K=8 broadcast selectors: out partition p gets rsum row 2j + (p >= 64)
    sel = np.zeros((4, 8, 128), np.float32)
    for j in range(4):
        sel[j, 2 * j, 0:64] = 1.0
        sel[j, 2 * j + 1, 64:128] = 1.0
    sel = sel.astype(bf16)

    in_maps = []
    for c in range(NCORES):
        b, r = c // 2, c % 2
        # xT packed: [ch, p, ko*512+t'] = x[b, 512ch+t', 128ko+p]
        xT = np.ascontiguousarray(
            x[b].reshape(4, 512, 8, 128).transpose(0, 3, 2, 1)
        ).reshape(4, 128, 4096).astype(bf16)
        qs, ks, vs = CL * r, C + CL * r, 2 * C + CL * r
        # waq/wak: [p, j, ko*128+n'] = W[:, 128j+n'] row 128ko+p
        waq = np.ascontiguousarray(
            W_attn[:, qs:qs + CL].reshape(8, 128, 4, 128).transpose(1, 2, 0, 3)
        ).reshape(128, 4, 1024).astype(bf16)
        wak = np.ascontiguousarray(
            W_attn[:, ks:ks + CL].reshape(8, 128, 4, 128).transpose(1, 2, 0, 3)
        ).reshape(128, 4, 1024).astype(bf16)
        wav = np.ascontiguousarray(
            W_attn[:, vs:vs + CL].reshape(8, 128, 512).transpose(1, 0, 2)
        ).astype(bf16)
        wp = np.ascontiguousarray(
            W_proj[CL * r:CL * (r + 1), :].reshape(4, 128, C).transpose(1, 0, 2)
        ).astype(bf16)
        bqk = np.empty((128, 8), np.float32)
        for j in range(4):
            bqk[:, j] = b_attn[qs + 128 * j:qs + 128 * (j + 1)]
            bqk[:, 4 + j] = b_attn[ks + 128 * j:ks + 128 * (j + 1)]
        in_maps.append({"xT": np.asarray(xT), "waq": np.asarray(waq),
                        "wak": np.asarray(wak), "wav": np.asarray(wav),
                        "wp": np.asarray(wp), "bqk": bqk,
                        "bye": np.asarray(bye), "um": np.asarray(um),
                        "sel": np.asarray(sel)})
    return in_maps


def kernel(x, W_attn, b_attn, W_proj, b_proj, _trace=False, _result=[None]):
    if "nc" not in _CACHE:
        _CACHE["nc"] = _build()
    nc = _CACHE["nc"]
    in_maps = _prep_inputs(x, W_attn, b_attn, W_proj, b_proj)
    res = run_bass_kernel_spmd(nc, in_maps, list(range(NCORES)), trace=_trace)
    _result[0] = res
    out = np.empty((B, T, C), np.float32)
    for b in range(B):
        ya = res.results[2 * b]["y"].astype(np.float32)
        yb = res.results[2 * b + 1]["y"].astype(np.float32)
        out[b] = (ya + yb).reshape(T, C)
    return out
